# revision 1
# baseline (speedup 1.0000x reference)
"""Trainium2 Bass kernel for nn_DentalAnatomyLoss.

Computes, for segmentation [B=2, C=32, D=64, H=128, W=128] fp32:
  - crown/root ratio loss (per (b,c) sums over d<32 / d>=32)
  - 3D total-variation loss (mean |diff| along w, h, d)
  - returns stack([crown_root, smoothness, total_anatomy]) fp32 [3]

Strategy: pure data-parallel over the 64 (b,c) slices, 8 per NeuronCore.
Each core reduces its 32 MiB shard to a [128, 52] fp32 partial tensor;
the host combines partials into the 3 scalars.

Layout: d-on-partitions. Each "chunk pair" (cp) holds 2 slices:
partition p = s*64 + d for local slice s in {0,1}, plane d in 0..63;
free axis = (h, w) = 16384 bf16. Benefits over the h-partition layout:
  - DMA reads are 16 KiB contiguous per partition (vs 512 B rows), and
    the fp32->bf16 cast happens inside the SWDGE DMA (measured at full
    HBM rate), freeing ScalarE entirely from casting.
  - The h-diff (gy) becomes an aligned free-axis shift by w -> one fused
    scalar_tensor_tensor max+accum per cp on VectorE.
  - The d-diff (gz) is the partition-axis diff -> TensorE block-bidiag
    matmul into PSUM (columns 63/127 zeroed so no cross-slice pairs),
    drained by ScalarE Abs+accum. Rows 63/127 drain |0| = 0.

Per-core engine budget (measured sustained rates):
  VectorE ~136 us: gx + gy fused STT max+accum (1x; sweeping 2x modes
    does not help: any elementwise+reduce pair costs the same 2 touches).
  ScalarE ~131 us: per-plane sum(x) via broadcast-out Copy+accum (fp32
    exact, feeds crown/root and the max-trick telescopes), PSUM drains,
    and the tiny first/last row/col telescope sums.
  TensorE ~70 us, DMA ~100 us (HBM roofline ~94 us/core).

Host recovers sum|a-b| = 2*sum(max(a,b)) - sum(a) - sum(b); the signed
sums telescope to per-plane sums and first/last row/col sums. gx and gy
share one denominator (d*h*(w-1) == d*(h-1)*w), gz has its own.
"""

import os

import numpy as np

B, C, D, H, W = 2, 32, 64, 128, 128
NCORES = 8
JPC = (B * C) // NCORES  # slices per core
CROWN_ROOT_W = 2.0
SMOOTH_W = 1.5
EXPECTED_RATIO = 1.2

# accumulator column layout in the [128, ACC_COLS] partial tensor.
# V and S ops run per quarter (nq=4 h-blocks per cp) so compute streams
# right behind each quarter's DMA; the host sums quarter columns.
NCP = JPC // 2  # chunk pairs per core
NQ = 4  # quarters per chunk pair
COL_SX = 0  # NCP*NQ: per-plane sum(x), per quarter
COL_GY = COL_SX + NCP * NQ  # NCP*NQ: per-plane sum(max h-pairs), in-quarter
COL_GX = COL_GY + NCP * NQ  # NCP*NQ: per-plane sum(max w-pairs), per quarter
COL_GYB = COL_GX + NCP * NQ  # NCP*(NQ-1): boundary-row max sums
COL_R = COL_GYB + NCP * (NQ - 1)  # NCP: per-plane sum(row0 + row_{h-1})
COL_C = COL_R + NCP  # NCP*NQ: per-plane sum(col0 + col_{w-1}), per quarter
COL_DZ = COL_C + NCP * NQ  # NCP*NDRAIN: PSUM |dz| drains
NDRAIN = 8
ACC_COLS = COL_DZ + NCP * NDRAIN

_PROG_CACHE: dict = {}
last_exec_time_ns = None


def _build_program(jpc=JPC, d=D, h=H, w=W, repeat=1, skip=()):
    """Build the (single) SPMD Bass program run identically on all cores.

    repeat>1 wraps the whole compute in a hardware For_i loop (identical
    result, used only for wall-clock timing of the kernel body).
    """
    from contextlib import ExitStack

    import concourse.tile as tile
    from concourse import bacc, mybir

    f32 = mybir.dt.float32
    bf16 = mybir.dt.bfloat16
    AO = mybir.AluOpType
    AF = mybir.ActivationFunctionType

    ncp = jpc // 2
    P = 2 * d  # partitions per chunk pair
    fsz = h * w  # free size per partition (one (h,w) plane)
    nq = 4  # DMA splits per chunk pair
    qsz = fsz // nq
    nblk = fsz // 512  # 512-col matmul blocks per cp
    ndrain = NDRAIN if fsz == 16384 else nq
    blk_per_drain = nblk // ndrain

    col_gyb = COL_GX + ncp * nq
    col_r = col_gyb + ncp * (nq - 1)
    col_c = col_r + ncp
    col_dz = col_c + ncp * nq
    acc_cols = col_dz + ncp * ndrain

    nc = bacc.Bacc(
        "TRN2",
        target_bir_lowering=False,
        debug=False,
        enable_asserts=False,
        num_devices=NCORES,
    )
    seg = nc.dram_tensor("seg", [jpc, d, h, w], f32, kind="ExternalInput").ap()
    bd = nc.dram_tensor("bidiag", [P, P], bf16, kind="ExternalInput").ap()
    out = nc.dram_tensor("partials", [P, acc_cols], f32, kind="ExternalOutput").ap()

    with tile.TileContext(nc) as tc, ExitStack() as ctx:
        singles = ctx.enter_context(tc.tile_pool(name="singles", bufs=1))
        xbp = ctx.enter_context(tc.tile_pool(name="xb", bufs=3))
        scrp = ctx.enter_context(tc.tile_pool(name="scr", bufs=2))
        dumbp = ctx.enter_context(tc.tile_pool(name="dumb", bufs=2))
        psp = ctx.enter_context(tc.tile_pool(name="ps", bufs=2, space="PSUM"))

        bd_sb = singles.tile([P, P], bf16)
        nc.sync.dma_start(out=bd_sb, in_=bd)
        acc = singles.tile([P, acc_cols], f32)
        nc.vector.memset(acc, 0.0)

        def cp_body(c):
            # 1) SWDGE cast-DMA loads: fp32 HBM -> bf16 SBUF, d-layout.
            #    Per partition: contiguous 4*qsz bytes from DRAM.
            xb = xbp.tile([P, fsz], bf16)
            src = seg[2 * c : 2 * c + 2].rearrange("s d h w -> (s d) (h w)")
            if "dma" not in skip:
                for q in range(nq):
                    nc.gpsimd.dma_start(
                        out=xb[:, q * qsz : (q + 1) * qsz],
                        in_=src[:, q * qsz : (q + 1) * qsz],
                    )
            elif "dma1" not in skip and c == 0:
                nc.gpsimd.dma_start(
                    out=xb[:, 0:qsz], in_=src[:, 0:qsz]
                )

            scratch = scrp.tile([P, qsz], bf16)
            dummy = dumbp.tile([P, 1], bf16)
            hq = qsz // w  # h-rows per quarter
            xb3 = xb.rearrange("p (r c2) -> p r c2", c2=w)
            scr3 = scratch.rearrange("p (r c2) -> p r c2", c2=w)

            # 2-4) Per-quarter compute, streamed behind each quarter's DMA:
            #   VectorE: fused max+accum for gy (shift by w, in-quarter) and
            #     gx (shift by 1 inside each w-row). Both 1x; one op each.
            #   ScalarE: per-plane sum(x) + col0/col_{w-1} telescope sums.
            #   TensorE block-bidiag d-diffs -> PSUM; ScalarE Abs drains.
            # Interior first/last-row telescope sums cancel against the
            # quarter-boundary terms, so only the cp-level row0+row_{h-1}
            # op and 3 tiny boundary-row max ops are needed.
            for q in range(nq):
                qc = nq * c + q
                r0 = q * hq
                if "gy" not in skip:
                    nc.vector.scalar_tensor_tensor(
                        out=scratch[:, 0 : qsz - w],
                        in0=xb[:, q * qsz + w : (q + 1) * qsz],
                        scalar=0.0,
                        in1=xb[:, q * qsz : (q + 1) * qsz - w],
                        op0=AO.bypass,
                        op1=AO.max,
                        accum_out=acc[:, COL_GY + qc : COL_GY + qc + 1],
                    )
                    if q > 0:
                        # boundary pair: last row of q-1, first row of q
                        nc.vector.scalar_tensor_tensor(
                            out=scr3[:, 0, :],
                            in0=xb3[:, r0, :],
                            scalar=0.0,
                            in1=xb3[:, r0 - 1, :],
                            op0=AO.bypass,
                            op1=AO.max,
                            accum_out=acc[
                                :,
                                col_gyb + (nq - 1) * c + q - 1 : col_gyb
                                + (nq - 1) * c
                                + q,
                            ],
                        )
                if "gx" not in skip:
                    nc.vector.scalar_tensor_tensor(
                        out=scr3[:, 0:hq, 0 : w - 1],
                        in0=xb3[:, r0 : r0 + hq, 1:w],
                        scalar=0.0,
                        in1=xb3[:, r0 : r0 + hq, 0 : w - 1],
                        op0=AO.bypass,
                        op1=AO.max,
                        accum_out=acc[:, COL_GX + qc : COL_GX + qc + 1],
                    )
                if "sx" not in skip:
                    nc.scalar.activation(
                        out=dummy.broadcast_to((P, qsz)),
                        in_=xb[:, q * qsz : (q + 1) * qsz],
                        func=AF.Copy,
                        accum_out=acc[:, COL_SX + qc : COL_SX + qc + 1],
                    )
                    cols = xb.rearrange("p (r c2) -> p c2 r", c2=w)[
                        :, 0 : w : w - 1, r0 : r0 + hq
                    ]
                    nc.scalar.activation(
                        out=dummy.broadcast_to((P, 2, hq)),
                        in_=cols,
                        func=AF.Copy,
                        accum_out=acc[:, col_c + qc : col_c + qc + 1],
                    )
                if "gz" not in skip:
                    dr_per_q = ndrain // nq
                    for t in range(dr_per_q):
                        ps = psp.tile([P, blk_per_drain, 512], f32)
                        for b in range(blk_per_drain):
                            blk = (q * dr_per_q + t) * blk_per_drain + b
                            nc.tensor.matmul(
                                ps[:, b, :],
                                bd_sb,
                                xb[:, blk * 512 : (blk + 1) * 512],
                                start=True,
                                stop=True,
                            )
                        col = col_dz + ndrain * c + q * dr_per_q + t
                        nc.scalar.activation(
                            out=dummy.broadcast_to((P, blk_per_drain, 512)),
                            in_=ps[:, :, :],
                            func=AF.Abs,
                            accum_out=acc[:, col : col + 1],
                        )
            if "sx" not in skip:
                # sum(row0 + row_{h-1}) per plane (cp-level; interior
                # quarter rows telescoped away)
                rows = xb3[:, 0 : h : h - 1, :]
                nc.scalar.activation(
                    out=dummy.broadcast_to((P, 2, w)),
                    in_=rows,
                    func=AF.Copy,
                    accum_out=acc[:, col_r + c : col_r + c + 1],
                )

        def all_cps():
            for c in range(ncp):
                cp_body(c)

        if repeat == 1:
            all_cps()
        else:
            with tc.For_i(0, repeat, 1):
                all_cps()
        nc.sync.dma_start(out=out, in_=acc)

    nc.compile()
    return nc


def _get_program():
    key = "full"
    if key not in _PROG_CACHE:
        _PROG_CACHE[key] = _build_program()
    return _PROG_CACHE[key]


def _bidiag_np(d=D):
    """lhsT for the d-diff matmul: out[m,:] = x[m+1,:] - x[m,:] within
    each slice; columns d-1 and 2d-1 zeroed (no cross-slice pairs)."""
    import ml_dtypes

    P = 2 * d
    m = np.zeros((P, P), dtype=np.float32)
    for col in range(P - 1):
        if col == d - 1:
            continue
        m[col, col] = -1.0
        m[col + 1, col] = 1.0
    return m.astype(ml_dtypes.bfloat16)


def _combine(partials, jpc=JPC, d=D, h=H, w=W):
    """Host-side finish: per-core [2d, acc_cols] fp32 partials -> [3]."""
    ncp = jpc // 2
    fsz = h * w
    nq = NQ
    nblk = fsz // 512
    ndrain = NDRAIN if fsz == 16384 else nq
    col_gyb = COL_GX + ncp * nq
    col_r = col_gyb + ncp * (nq - 1)
    col_c = col_r + ncp
    col_dz = col_c + ncp * nq

    nslice = jpc * len(partials)
    crown = np.zeros(nslice, dtype=np.float64)
    root = np.zeros(nslice, dtype=np.float64)
    gxy_sum = 0.0
    gz_sum = 0.0
    for k, p in enumerate(partials):
        p = p.astype(np.float64)
        for c in range(ncp):
            qs = slice(nq * c, nq * c + nq)
            sx = p[:, COL_SX + nq * c : COL_SX + nq * c + nq].sum(axis=1)
            gy = p[:, COL_GY + nq * c : COL_GY + nq * c + nq].sum(axis=1)
            gyb = p[
                :, col_gyb + (nq - 1) * c : col_gyb + (nq - 1) * c + nq - 1
            ].sum(axis=1)
            gx = p[:, COL_GX + nq * c : COL_GX + nq * c + nq].sum(axis=1)
            rr = p[:, col_r + c]  # per-plane sum(row0 + row_{h-1})
            cc = p[:, col_c + nq * c : col_c + nq * c + nq].sum(axis=1)
            # sum|a-b| = 2*sum(max) - sum(a) - sum(b); the signed sums
            # telescope: gy: -2*sx + rr ; gx: -2*sx + cc (per plane).
            # In-quarter gy maxes + boundary-row maxes cover all h-pairs.
            gxy_sum += (2.0 * (gy + gyb) - 2.0 * sx + rr).sum()
            gxy_sum += (2.0 * gx - 2.0 * sx + cc).sum()
            for s in (0, 1):
                sl = k * jpc + 2 * c + s
                crown[sl] = sx[s * d : s * d + d // 2].sum()
                root[sl] = sx[s * d + d // 2 : s * d + d].sum()
        dz = p[:, col_dz : col_dz + ncp * ndrain]
        # rows d-1 and 2d-1 are |0| = 0 (zeroed bidiag columns)
        gz_sum += dz.sum()

    total = crown + root
    valid = (total > 0) & (root > 0)
    safe_root = np.where(root > 0, root, 1.0)
    ratio_loss = np.where(valid, (crown / safe_root - EXPECTED_RATIO) ** 2, 0.0)
    cr_loss = ratio_loss.sum() / nslice

    nxy = nslice * d * h * (w - 1)  # == nslice * d * (h-1) * w
    nz = nslice * (d - 1) * h * w
    tv = gxy_sum / nxy + gz_sum / nz

    crown_root = cr_loss * CROWN_ROOT_W
    smoothness = tv * SMOOTH_W
    return np.array(
        [crown_root, smoothness, crown_root + smoothness], dtype=np.float32
    )


def kernel(segmentation: np.ndarray) -> np.ndarray:
    global last_exec_time_ns
    from concourse.bass_utils import run_bass_kernel_spmd

    seg = np.ascontiguousarray(np.asarray(segmentation), dtype=np.float32)
    assert seg.shape == (B, C, D, H, W)
    nc = _get_program()

    bd = _bidiag_np()
    shards = seg.reshape(B * C, D, H, W)
    in_maps = [
        {"seg": np.ascontiguousarray(shards[k * JPC : (k + 1) * JPC]), "bidiag": bd}
        for k in range(NCORES)
    ]
    trace = bool(os.environ.get("BASS_TRACE"))
    res = run_bass_kernel_spmd(nc, in_maps, list(range(NCORES)), trace=trace)
    last_exec_time_ns = res.exec_time_ns
    partials = [res.results[k]["partials"] for k in range(NCORES)]
    return _combine(partials)



# revision 6
# speedup vs baseline: 1.1518x; 1.1518x over previous
"""Trainium2 Bass kernel for nn_DentalAnatomyLoss (v2).

Computes, for segmentation [B=2, C=32, D=64, H=128, W=128] fp32:
  - crown/root ratio loss (per (b,c) sums over d<32 / d>=32)
  - 3D total-variation loss (mean |diff| along w, h, d)
  - returns stack([crown_root, smoothness, total_anatomy]) fp32 [3]

Pure data-parallel over the 64 (b,c) slices, 8 per NeuronCore. Each
core reduces its 32 MiB shard to a [128, ACC] fp32 partial tensor; the
host combines partials into the 3 scalars.

Layout: d-on-partitions, 2 slices per chunk pair (cp): partition
p = s*64 + d; free = (h, w) = 16384 bf16 (fp32->bf16 cast inside the
SWDGE DMA). sum|a-b| = 2*sum(max(a,b)) - sum(a) - sum(b); the signed
sums telescope to per-plane/edge sums.

v2 engine assignment (vs v1's fused scalar_tensor_tensor at 1x):
  VectorE: gy/gx max via tensor_tensor(max) (2x_1p, bf16) into SBUF
    scratch + gx scratch reduce via tensor_scalar+accum (4x_2p).
    ~88 us/core (vs 136 at 1x).
  TensorE: block-bidiag d-diff matmuls into PSUM, PLUS free-axis
    reductions via accumulating matmuls: ones-row reduce of the gy
    scratch and a [128,4] group-selector matmul of x (crown/root per
    slice). ~82 us/core.
  ScalarE: PSUM |dz| drains (Abs+accum), tiny edge row/col sums, and
    per-cp drains of the accumulating PSUM rows. ~77 us/core.
  DMA: ~94-100 us/core (HBM roofline ~358 GB/s/core) -> the target.

gy pairs for quarter q<3 read w elements into quarter q+1 (emitted
after that DMA), so no boundary ops and no gyb telescope terms.
"""

import os

import numpy as np

B, C, D, H, W = 2, 32, 64, 128, 128
NCORES = 8
JPC = (B * C) // NCORES  # slices per core
CROWN_ROOT_W = 2.0
SMOOTH_W = 1.5
EXPECTED_RATIO = 1.2

NQ = 4  # DMA quarters per chunk pair
GRP = (3, 3, 2)  # diff-PSUM group sizes (blocks) per quarter
GY_ROW = 32  # PSUM partition row of the gy-reduce accumulation

_PROG_CACHE: dict = {}
last_exec_time_ns = None


def _layout(ncp):
    """acc column layout for the [128, acc_cols] fp32 partial tensor."""
    ndr = len(GRP) * NQ  # diff drains per cp
    col_mx = 0  # ncp*nq: per-plane sum(max w-pairs), per quarter
    col_r = col_mx + ncp * NQ  # ncp: per-plane sum(row0 + row_{h-1})
    col_c = col_r + ncp  # ncp: per-plane sum(col0 + col_{w-1})
    col_ps = col_c + ncp  # ncp: rows 0..3 = crown/root sums, row 32 = My
    col_dz = col_ps + ncp  # ncp*ndr: PSUM |dz| drains
    acc_cols = col_dz + ncp * ndr
    return ndr, col_mx, col_r, col_c, col_ps, col_dz, acc_cols


def _build_program(jpc=JPC, d=D, h=H, w=W, repeat=1, skip=()):
    """Build the (single) SPMD Bass program run identically on all cores.

    repeat>1 wraps the whole compute in a hardware For_i loop (identical
    result, used only for wall-clock timing of the kernel body).
    skip=(...) ablates op groups for engine-time attribution.
    """
    from contextlib import ExitStack

    import concourse.tile as tile
    from concourse import bacc, mybir

    f32 = mybir.dt.float32
    bf16 = mybir.dt.bfloat16
    AO = mybir.AluOpType
    AF = mybir.ActivationFunctionType

    ncp = jpc // 2
    P = 2 * d  # partitions per chunk pair
    fsz = h * w  # free size per partition (one (h,w) plane)
    qsz = fsz // NQ
    hq = h // NQ  # h-rows per quarter
    nblk = fsz // 512  # 512-col matmul blocks per cp
    bq = nblk // NQ  # blocks per quarter
    assert sum(GRP) == bq
    ndr, col_mx, col_r, col_c, col_ps, col_dz, acc_cols = _layout(ncp)

    nc = bacc.Bacc(
        "TRN2",
        target_bir_lowering=False,
        debug=False,
        enable_asserts=False,
        num_devices=NCORES,
    )
    seg = nc.dram_tensor("seg", [jpc, d, h, w], f32, kind="ExternalInput").ap()
    aux = nc.dram_tensor("aux", [P, P + 5], bf16, kind="ExternalInput").ap()
    out = nc.dram_tensor("partials", [P, acc_cols], f32, kind="ExternalOutput").ap()

    with tile.TileContext(nc) as tc, ExitStack() as ctx:
        singles = ctx.enter_context(tc.tile_pool(name="singles", bufs=1))
        xbp = ctx.enter_context(tc.tile_pool(name="xb", bufs=3))
        gyp = ctx.enter_context(tc.tile_pool(name="gy", bufs=3))
        gxp = ctx.enter_context(tc.tile_pool(name="gx", bufs=3))
        psp = ctx.enter_context(tc.tile_pool(name="ps", bufs=2, space="PSUM"))
        accp = ctx.enter_context(tc.tile_pool(name="accps", bufs=2, space="PSUM"))

        aux_sb = singles.tile([P, P + 5], bf16)
        nc.sync.dma_start(out=aux_sb, in_=aux)
        bd_ap = aux_sb[:, 0:P]  # block-bidiag d-diff lhsT
        xsel_ap = aux_sb[:, P : P + 4]  # crown/root group selector lhsT
        ones_ap = aux_sb[:, P + 4 : P + 5]  # gy-reduce lhsT

        acc = singles.tile([P, acc_cols], f32)
        nc.vector.memset(acc, 0.0)
        dump = singles.tile([P, qsz], bf16)  # VectorE reduce out sink
        sdump = singles.tile([P, 512], bf16)  # ScalarE psum-drain out sink
        dummy = singles.tile([P, 1], bf16)  # ScalarE broadcast out sink

        def emit_gyred(c, q, gy_t, fd, acc_ps):
            # free-axis reduce of the gy max scratch: accumulate column
            # sums into PSUM row GY_ROW across all chunks of the cp.
            nchunks = (fd + 511) // 512
            for k in range(nchunks):
                c0 = k * 512
                csz = min(512, fd - c0)
                nc.tensor.matmul(
                    acc_ps[GY_ROW : GY_ROW + 1, 0:csz],
                    ones_ap,
                    gy_t[:, c0 : c0 + csz],
                    start=(q == 0 and k == 0),
                    stop=(q == NQ - 1 and k == nchunks - 1),
                    skip_group_check=True,
                )

        def cp_body(c):
            xb = xbp.tile([P, fsz], bf16)
            src = seg[2 * c : 2 * c + 2].rearrange("s d h w -> (s d) (h w)")
            xb3 = xb.rearrange("p (r c2) -> p r c2", c2=w)
            acc_ps = accp.tile([P, 512], f32)
            gy_tiles = []

            def emit_gy(q):
                # h-pairs for rows q*hq .. q*hq+hq-1; q<3 reads w elements
                # into quarter q+1 (its DMA precedes this op).
                fd = qsz if q < NQ - 1 else qsz - w
                gy_t = gyp.tile([P, qsz], bf16)
                nc.vector.tensor_max(
                    gy_t[:, 0:fd],
                    xb[:, q * qsz : q * qsz + fd],
                    xb[:, q * qsz + w : q * qsz + w + fd],
                )
                gy_tiles.append((gy_t, fd))

            for q in range(NQ):
                if "dma" not in skip:
                    nc.gpsimd.dma_start(
                        out=xb[:, q * qsz : (q + 1) * qsz],
                        in_=src[:, q * qsz : (q + 1) * qsz],
                    )
                r0 = q * hq
                # --- VectorE: w-pair maxes (2x) + scratch reduce (4x) ---
                if "gx" not in skip:
                    gx_t = gxp.tile([P, hq * (w - 1)], bf16)
                    gx3 = gx_t.rearrange("p (r c2) -> p r c2", c2=w - 1)
                    nc.vector.tensor_max(
                        gx3[:, :, :],
                        xb3[:, r0 : r0 + hq, 0 : w - 1],
                        xb3[:, r0 : r0 + hq, 1:w],
                    )
                    if "gxred" not in skip:
                        colq = col_mx + NQ * c + q
                        nc.vector.tensor_scalar(
                            out=dump[:, 0 : hq * (w - 1)],
                            in0=gx_t[:, :],
                            scalar1=0.0,
                            scalar2=0.0,
                            op0=AO.bypass,
                            op1=AO.add,
                            accum_out=acc[:, colq : colq + 1],
                        )
                if "gy" not in skip and q > 0:
                    emit_gy(q - 1)
                    if q == NQ - 1:
                        emit_gy(q)
                # --- TensorE: d-diff + group-sum matmuls; ScalarE drains ---
                for g, gsz in enumerate(GRP):
                    goff = q * bq + sum(GRP[:g])
                    if "gz" not in skip:
                        # constant-size PSUM tile (uniform pool slots); the
                        # last group only uses gsz of the GRP[0] blocks
                        ps = psp.tile([P, GRP[0], 512], f32)
                        for j in range(gsz):
                            blk = goff + j
                            nc.tensor.matmul(
                                ps[:, j, :],
                                bd_ap,
                                xb[:, blk * 512 : (blk + 1) * 512],
                                start=True,
                                stop=True,
                            )
                    if "xs" not in skip:
                        for j in range(gsz):
                            blk = goff + j
                            nc.tensor.matmul(
                                acc_ps[0:4, :],
                                xsel_ap,
                                xb[:, blk * 512 : (blk + 1) * 512],
                                start=(blk == 0),
                                stop=(blk == nblk - 1),
                                skip_group_check=True,
                            )
                    if "gz" not in skip and "drain" not in skip:
                        colx = col_dz + ndr * c + len(GRP) * q + g
                        nc.scalar.activation(
                            out=dummy.broadcast_to((P, gsz, 512)),
                            in_=ps[:, 0:gsz, :],
                            func=AF.Abs,
                            accum_out=acc[:, colx : colx + 1],
                        )
                # --- TensorE: gy scratch reduce (lags gy by <=1 quarter) ---
                if "gy" not in skip and "gyred" not in skip:
                    if q > 0:
                        emit_gyred(c, q - 1, *gy_tiles[q - 1], acc_ps)
                        if q == NQ - 1:
                            emit_gyred(c, q, *gy_tiles[q], acc_ps)

            # --- ScalarE: edge sums + accumulating-PSUM drains ---
            if "edges" not in skip:
                rows = xb3[:, 0 : h : h - 1, :]
                nc.scalar.activation(
                    out=dummy.broadcast_to((P, 2, w)),
                    in_=rows,
                    func=AF.Copy,
                    accum_out=acc[:, col_r + c : col_r + c + 1],
                )
                colsv = xb.rearrange("p (r c2) -> p c2 r", c2=w)[:, 0 : w : w - 1, :]
                nc.scalar.activation(
                    out=dummy.broadcast_to((P, 2, h)),
                    in_=colsv,
                    func=AF.Copy,
                    accum_out=acc[:, col_c + c : col_c + c + 1],
                )
            if "xs" not in skip:
                nc.scalar.activation(
                    out=sdump[0:4, 0:512],
                    in_=acc_ps[0:4, :],
                    func=AF.Copy,
                    accum_out=acc[0:4, col_ps + c : col_ps + c + 1],
                )
            if "gy" not in skip and "gyred" not in skip:
                nc.scalar.activation(
                    out=sdump[GY_ROW : GY_ROW + 1, 0:512],
                    in_=acc_ps[GY_ROW : GY_ROW + 1, :],
                    func=AF.Copy,
                    accum_out=acc[GY_ROW : GY_ROW + 1, col_ps + c : col_ps + c + 1],
                )

        def all_cps():
            for c in range(ncp):
                cp_body(c)

        if repeat == 1:
            all_cps()
        else:
            with tc.For_i(0, repeat, 1):
                all_cps()
        nc.sync.dma_start(out=out, in_=acc)

    nc.compile()
    return nc


def _get_program():
    key = "full"
    if key not in _PROG_CACHE:
        _PROG_CACHE[key] = _build_program()
    return _PROG_CACHE[key]


def _aux_np(d=D):
    """[2d, 2d+5] bf16 lhsT bundle: cols 0..2d-1 block-bidiag (out row m =
    x[m+1]-x[m] within each slice; cols d-1, 2d-1 zero), cols 2d..2d+3
    crown/root group selectors, col 2d+4 ones (gy reduce)."""
    import ml_dtypes

    P = 2 * d
    a = np.zeros((P, P + 5), dtype=np.float32)
    for col in range(P - 1):
        if col == d - 1:
            continue
        a[col, col] = -1.0
        a[col + 1, col] = 1.0
    hd = d // 2
    for j in range(4):
        a[j * hd : (j + 1) * hd, P + j] = 1.0
    a[:, P + 4] = 1.0
    return a.astype(ml_dtypes.bfloat16)


def _combine(partials, jpc=JPC, d=D, h=H, w=W):
    """Host-side finish: per-core [2d, acc_cols] fp32 partials -> [3]."""
    ncp = jpc // 2
    ndr, col_mx, col_r, col_c, col_ps, col_dz, acc_cols = _layout(ncp)

    nslice = jpc * len(partials)
    crown = np.zeros(nslice, dtype=np.float64)
    root = np.zeros(nslice, dtype=np.float64)
    gxy_sum = 0.0
    gz_sum = 0.0
    for k, p in enumerate(partials):
        p = p.astype(np.float64)
        for c in range(ncp):
            cr0, rt0, cr1, rt1 = p[0:4, col_ps + c]
            my = p[GY_ROW, col_ps + c]
            s_cp = cr0 + rt0 + cr1 + rt1
            r_cp = p[:, col_r + c].sum()
            c_cp = p[:, col_c + c].sum()
            mx = p[:, col_mx + NQ * c : col_mx + NQ * c + NQ].sum()
            # sum|a-b| = 2*sum(max) - sum(a) - sum(b); signed sums telescope
            gxy_sum += 2.0 * my - 2.0 * s_cp + r_cp
            gxy_sum += 2.0 * mx - 2.0 * s_cp + c_cp
            sl = k * jpc + 2 * c
            crown[sl], root[sl] = cr0, rt0
            crown[sl + 1], root[sl + 1] = cr1, rt1
        # diff rows d-1 and 2d-1 are |0| = 0 (zeroed bidiag columns)
        gz_sum += p[:, col_dz : col_dz + ncp * ndr].sum()

    total = crown + root
    valid = (total > 0) & (root > 0)
    safe_root = np.where(root > 0, root, 1.0)
    ratio_loss = np.where(valid, (crown / safe_root - EXPECTED_RATIO) ** 2, 0.0)
    cr_loss = ratio_loss.sum() / nslice

    nxy = nslice * d * h * (w - 1)  # == nslice * d * (h-1) * w
    nz = nslice * (d - 1) * h * w
    tv = gxy_sum / nxy + gz_sum / nz

    crown_root = cr_loss * CROWN_ROOT_W
    smoothness = tv * SMOOTH_W
    return np.array(
        [crown_root, smoothness, crown_root + smoothness], dtype=np.float32
    )


def kernel(segmentation: np.ndarray) -> np.ndarray:
    global last_exec_time_ns
    from concourse.bass_utils import run_bass_kernel_spmd

    seg = np.ascontiguousarray(np.asarray(segmentation), dtype=np.float32)
    assert seg.shape == (B, C, D, H, W)
    nc = _get_program()

    aux = _aux_np()
    shards = seg.reshape(B * C, D, H, W)
    in_maps = [
        {"seg": np.ascontiguousarray(shards[k * JPC : (k + 1) * JPC]), "aux": aux}
        for k in range(NCORES)
    ]
    trace = bool(os.environ.get("BASS_TRACE"))
    res = run_bass_kernel_spmd(nc, in_maps, list(range(NCORES)), trace=trace)
    last_exec_time_ns = res.exec_time_ns
    partials = [res.results[k]["partials"] for k in range(NCORES)]
    return _combine(partials)


# revision 14
# speedup vs baseline: 1.3087x; 1.1362x over previous
"""Trainium2 Bass kernel for nn_DentalAnatomyLoss (v2).

Computes, for segmentation [B=2, C=32, D=64, H=128, W=128] fp32:
  - crown/root ratio loss (per (b,c) sums over d<32 / d>=32)
  - 3D total-variation loss (mean |diff| along w, h, d)
  - returns stack([crown_root, smoothness, total_anatomy]) fp32 [3]

Pure data-parallel over the 64 (b,c) slices, 8 per NeuronCore. Each
core reduces its 32 MiB shard to a [128, ACC] fp32 partial tensor; the
host combines partials into the 3 scalars.

Layout: d-on-partitions, 2 slices per chunk pair (cp): partition
p = s*64 + d; free = (h, w) = 16384 bf16 (fp32->bf16 cast inside the
SWDGE DMA). sum|a-b| = 2*sum(max(a,b)) - sum(a) - sum(b); the signed
sums telescope to per-plane/edge sums.

v2 engine assignment (vs v1's fused scalar_tensor_tensor at 1x):
  VectorE: gy/gx max via tensor_tensor(max) (2x_1p, bf16) into SBUF
    scratch + gx scratch reduce via tensor_scalar+accum (4x_2p).
    ~88 us/core (vs 136 at 1x).
  TensorE: block-bidiag d-diff matmuls into PSUM, PLUS free-axis
    reductions via accumulating matmuls: ones-row reduce of the gy
    scratch and a [128,4] group-selector matmul of x (crown/root per
    slice). ~82 us/core.
  ScalarE: PSUM |dz| drains (Abs+accum), tiny edge row/col sums, and
    per-cp drains of the accumulating PSUM rows. ~77 us/core.
  DMA: ~94-100 us/core (HBM roofline ~358 GB/s/core) -> the target.

gy pairs for quarter q<3 read w elements into quarter q+1 (emitted
after that DMA), so no boundary ops and no gyb telescope terms.
"""

import os

import numpy as np

B, C, D, H, W = 2, 32, 64, 128, 128
NCORES = 8
JPC = (B * C) // NCORES  # slices per core
CROWN_ROOT_W = 2.0
SMOOTH_W = 1.5
EXPECTED_RATIO = 1.2

NQ = 4  # DMA quarters per chunk pair
GRP = (3, 3, 2)  # diff-PSUM group sizes (blocks) per quarter
GY_ROW = 32  # PSUM partition row of the gy-reduce accumulation

_PROG_CACHE: dict = {}
last_exec_time_ns = None


def _layout(ncp):
    """acc column layout for the [128, acc_cols] fp32 partial tensor."""
    ndr = len(GRP) * NQ  # diff drains per cp
    col_mx = 0  # ncp*nq: per-plane sum(max w-pairs), per quarter
    col_r = col_mx + ncp * NQ  # ncp: per-plane sum(row0 + row_{h-1})
    col_c = col_r + ncp  # ncp: per-plane sum(col0 + col_{w-1})
    col_ps = col_c + ncp  # ncp: rows 0..3 = crown/root sums, row 32 = My
    col_dz = col_ps + ncp  # ncp*ndr: PSUM |dz| drains
    acc_cols = col_dz + ncp * ndr
    return ndr, col_mx, col_r, col_c, col_ps, col_dz, acc_cols


def _build_program(jpc=JPC, d=D, h=H, w=W, repeat=1, skip=()):
    """Build the (single) SPMD Bass program run identically on all cores.

    repeat>1 wraps the whole compute in a hardware For_i loop (identical
    result, used only for wall-clock timing of the kernel body).
    skip=(...) ablates op groups for engine-time attribution.
    """
    from contextlib import ExitStack

    import concourse.tile as tile
    from concourse import bacc, mybir

    f32 = mybir.dt.float32
    bf16 = mybir.dt.bfloat16
    AO = mybir.AluOpType
    AF = mybir.ActivationFunctionType

    ncp = jpc // 2
    P = 2 * d  # partitions per chunk pair
    fsz = h * w  # free size per partition (one (h,w) plane)
    qsz = fsz // NQ
    hq = h // NQ  # h-rows per quarter
    nblk = fsz // 512  # 512-col matmul blocks per cp
    bq = nblk // NQ  # blocks per quarter
    assert sum(GRP) == bq
    ndr, col_mx, col_r, col_c, col_ps, col_dz, acc_cols = _layout(ncp)

    nc = bacc.Bacc(
        "TRN2",
        target_bir_lowering=False,
        debug=False,
        enable_asserts=False,
        num_devices=NCORES,
    )
    seg = nc.dram_tensor("seg", [jpc, d, h, w], f32, kind="ExternalInput").ap()
    aux = nc.dram_tensor("aux", [P, P + 5], bf16, kind="ExternalInput").ap()
    out = nc.dram_tensor("partials", [P, acc_cols], f32, kind="ExternalOutput").ap()

    with tile.TileContext(nc) as tc, ExitStack() as ctx:
        singles = ctx.enter_context(tc.tile_pool(name="singles", bufs=1))
        xbp = ctx.enter_context(tc.tile_pool(name="xb", bufs=3))
        gyp = ctx.enter_context(tc.tile_pool(name="gy", bufs=3))
        psp = ctx.enter_context(tc.tile_pool(name="ps", bufs=2, space="PSUM"))
        accp = ctx.enter_context(tc.tile_pool(name="accps", bufs=2, space="PSUM"))

        aux_sb = singles.tile([P, P + 5], bf16)
        nc.sync.dma_start(out=aux_sb, in_=aux)
        bd_ap = aux_sb[:, 0:P]  # block-bidiag d-diff lhsT
        xsel_ap = aux_sb[:, P : P + 4]  # crown/root group selector lhsT
        ones_ap = aux_sb[:, P + 4 : P + 5]  # gy-reduce lhsT

        acc = singles.tile([P, acc_cols], f32)
        nc.vector.memset(acc, 0.0)
        dump = singles.tile([P, (h // 2) * (w - 1)], bf16)  # VectorE out sink
        sdump = singles.tile([P, 512], bf16)  # ScalarE psum-drain out sink
        dummy = singles.tile([P, 1], bf16)  # ScalarE broadcast out sink

        def emit_gyred(c, half, gy_t, fd, acc_ps):
            # free-axis reduce of the gy max scratch: accumulate column
            # sums into PSUM row GY_ROW across all chunks of the cp.
            nchunks = (fd + 511) // 512
            for k in range(nchunks):
                c0 = k * 512
                csz = min(512, fd - c0)
                nc.tensor.matmul(
                    acc_ps[GY_ROW : GY_ROW + 1, 0:csz],
                    ones_ap,
                    gy_t[:, c0 : c0 + csz],
                    start=(half == 0 and k == 0),
                    stop=(half == 1 and k == nchunks - 1),
                    skip_group_check=True,
                )

        def cp_body(c):
            xb = xbp.tile([P, fsz], bf16)
            src = seg[2 * c : 2 * c + 2].rearrange("s d h w -> (s d) (h w)")
            xb3 = xb.rearrange("p (r c2) -> p r c2", c2=w)
            acc_ps = accp.tile([P, 512], f32)
            gy_tiles = []

            hsz = fsz // 2

            def emit_gy(half):
                # h-pairs for rows of this half-cp; half 0 reads w elements
                # into the next half (its DMA precedes this op).
                fd = hsz if half == 0 else hsz - w
                gy_t = gyp.tile([P, hsz], bf16)
                nc.vector.tensor_max(
                    gy_t[:, 0:fd],
                    xb[:, half * hsz : half * hsz + fd],
                    xb[:, half * hsz + w : half * hsz + w + fd],
                )
                gy_tiles.append((gy_t, fd))

            def emit_gx(half):
                # fused max+accum over w-pairs (1x STT, but no scratch
                # reduce stream needed on T/V)
                r0 = half * (h // 2)
                colq = col_mx + NQ * c + half
                nc.vector.scalar_tensor_tensor(
                    out=dump.rearrange("p (r c2) -> p r c2", c2=w - 1)[
                        :, 0 : h // 2, :
                    ],
                    in0=xb3[:, r0 : r0 + h // 2, 1:w],
                    scalar=0.0,
                    in1=xb3[:, r0 : r0 + h // 2, 0 : w - 1],
                    op0=AO.bypass,
                    op1=AO.max,
                    accum_out=acc[:, colq : colq + 1],
                )

            for q in range(NQ):
                if "dma" not in skip:
                    nc.gpsimd.dma_start(
                        out=xb[:, q * qsz : (q + 1) * qsz],
                        in_=src[:, q * qsz : (q + 1) * qsz],
                    )
                elif q == 0:
                    # timing ablation: tiny write so the tile allocates
                    nc.gpsimd.dma_start(out=xb[:, 0:512], in_=src[:, 0:512])
                # --- VectorE: half-cp ops; gx fused STT, gy tt-max (2x) ---
                if "gx" not in skip and q in (1, NQ - 1):
                    emit_gx(0 if q == 1 else 1)
                if "gy" not in skip and q >= 2:
                    emit_gy(0 if q == 2 else 1)
                # --- TensorE: d-diff + group-sum matmuls; ScalarE drains ---
                for g, gsz in enumerate(GRP):
                    goff = q * bq + sum(GRP[:g])
                    if "gz" not in skip:
                        # constant-size PSUM tile (uniform pool slots); the
                        # last group only uses gsz of the GRP[0] blocks
                        ps = psp.tile([P, GRP[0], 512], f32)
                        for j in range(gsz):
                            blk = goff + j
                            nc.tensor.matmul(
                                ps[:, j, :],
                                bd_ap,
                                xb[:, blk * 512 : (blk + 1) * 512],
                                start=True,
                                stop=True,
                            )
                    if "xs" not in skip:
                        for j in range(gsz):
                            blk = goff + j
                            nc.tensor.matmul(
                                acc_ps[0:4, :],
                                xsel_ap,
                                xb[:, blk * 512 : (blk + 1) * 512],
                                start=(blk == 0),
                                stop=(blk == nblk - 1),
                                skip_group_check=True,
                            )
                    if "gz" not in skip and "drain" not in skip:
                        colx = col_dz + ndr * c + len(GRP) * q + g
                        nc.scalar.activation(
                            out=dummy.broadcast_to((P, gsz, 512)),
                            in_=ps[:, 0:gsz, :],
                            func=AF.Abs,
                            accum_out=acc[:, colx : colx + 1],
                        )
                # --- TensorE: gy scratch reduce (after the V max ops) ---
                if "gy" not in skip and "gyred" not in skip and q == NQ - 1:
                    emit_gyred(c, 0, *gy_tiles[0], acc_ps)
                    emit_gyred(c, 1, *gy_tiles[1], acc_ps)

            # --- ScalarE: edge sums + accumulating-PSUM drains ---
            if "edges" not in skip:
                rows = xb3[:, 0 : h : h - 1, :]
                nc.scalar.activation(
                    out=dummy.broadcast_to((P, 2, w)),
                    in_=rows,
                    func=AF.Copy,
                    accum_out=acc[:, col_r + c : col_r + c + 1],
                )
                colsv = xb.rearrange("p (r c2) -> p c2 r", c2=w)[:, 0 : w : w - 1, :]
                nc.scalar.activation(
                    out=dummy.broadcast_to((P, 2, h)),
                    in_=colsv,
                    func=AF.Copy,
                    accum_out=acc[:, col_c + c : col_c + c + 1],
                )
            if "xs" not in skip:
                nc.scalar.activation(
                    out=sdump[0:4, 0:512],
                    in_=acc_ps[0:4, :],
                    func=AF.Copy,
                    accum_out=acc[0:4, col_ps + c : col_ps + c + 1],
                )
            if "gy" not in skip and "gyred" not in skip:
                nc.scalar.activation(
                    out=sdump[GY_ROW : GY_ROW + 1, 0:512],
                    in_=acc_ps[GY_ROW : GY_ROW + 1, :],
                    func=AF.Copy,
                    accum_out=acc[GY_ROW : GY_ROW + 1, col_ps + c : col_ps + c + 1],
                )

        def all_cps():
            for c in range(ncp):
                cp_body(c)

        if repeat == 1:
            all_cps()
        else:
            with tc.For_i(0, repeat, 1):
                all_cps()
        nc.sync.dma_start(out=out, in_=acc)

    nc.compile()
    return nc


def _get_program():
    key = "full"
    if key not in _PROG_CACHE:
        _PROG_CACHE[key] = _build_program()
    return _PROG_CACHE[key]


def _aux_np(d=D):
    """[2d, 2d+5] bf16 lhsT bundle: cols 0..2d-1 block-bidiag (out row m =
    x[m+1]-x[m] within each slice; cols d-1, 2d-1 zero), cols 2d..2d+3
    crown/root group selectors, col 2d+4 ones (gy reduce)."""
    import ml_dtypes

    P = 2 * d
    a = np.zeros((P, P + 5), dtype=np.float32)
    for col in range(P - 1):
        if col == d - 1:
            continue
        a[col, col] = -1.0
        a[col + 1, col] = 1.0
    hd = d // 2
    for j in range(4):
        a[j * hd : (j + 1) * hd, P + j] = 1.0
    a[:, P + 4] = 1.0
    return a.astype(ml_dtypes.bfloat16)


def _combine(partials, jpc=JPC, d=D, h=H, w=W):
    """Host-side finish: per-core [2d, acc_cols] fp32 partials -> [3]."""
    ncp = jpc // 2
    ndr, col_mx, col_r, col_c, col_ps, col_dz, acc_cols = _layout(ncp)

    nslice = jpc * len(partials)
    crown = np.zeros(nslice, dtype=np.float64)
    root = np.zeros(nslice, dtype=np.float64)
    gxy_sum = 0.0
    gz_sum = 0.0
    for k, p in enumerate(partials):
        p = p.astype(np.float64)
        for c in range(ncp):
            cr0, rt0, cr1, rt1 = p[0:4, col_ps + c]
            my = p[GY_ROW, col_ps + c]
            s_cp = cr0 + rt0 + cr1 + rt1
            r_cp = p[:, col_r + c].sum()
            c_cp = p[:, col_c + c].sum()
            mx = p[:, col_mx + NQ * c : col_mx + NQ * c + 2].sum()
            # sum|a-b| = 2*sum(max) - sum(a) - sum(b); signed sums telescope
            gxy_sum += 2.0 * my - 2.0 * s_cp + r_cp
            gxy_sum += 2.0 * mx - 2.0 * s_cp + c_cp
            sl = k * jpc + 2 * c
            crown[sl], root[sl] = cr0, rt0
            crown[sl + 1], root[sl + 1] = cr1, rt1
        # diff rows d-1 and 2d-1 are |0| = 0 (zeroed bidiag columns)
        gz_sum += p[:, col_dz : col_dz + ncp * ndr].sum()

    total = crown + root
    valid = (total > 0) & (root > 0)
    safe_root = np.where(root > 0, root, 1.0)
    ratio_loss = np.where(valid, (crown / safe_root - EXPECTED_RATIO) ** 2, 0.0)
    cr_loss = ratio_loss.sum() / nslice

    nxy = nslice * d * h * (w - 1)  # == nslice * d * (h-1) * w
    nz = nslice * (d - 1) * h * w
    tv = gxy_sum / nxy + gz_sum / nz

    crown_root = cr_loss * CROWN_ROOT_W
    smoothness = tv * SMOOTH_W
    return np.array(
        [crown_root, smoothness, crown_root + smoothness], dtype=np.float32
    )


def kernel(segmentation: np.ndarray) -> np.ndarray:
    global last_exec_time_ns
    from concourse.bass_utils import run_bass_kernel_spmd

    seg = np.ascontiguousarray(np.asarray(segmentation), dtype=np.float32)
    assert seg.shape == (B, C, D, H, W)
    nc = _get_program()

    aux = _aux_np()
    shards = seg.reshape(B * C, D, H, W)
    in_maps = [
        {"seg": np.ascontiguousarray(shards[k * JPC : (k + 1) * JPC]), "aux": aux}
        for k in range(NCORES)
    ]
    trace = bool(os.environ.get("BASS_TRACE"))
    res = run_bass_kernel_spmd(nc, in_maps, list(range(NCORES)), trace=trace)
    last_exec_time_ns = res.exec_time_ns
    partials = [res.results[k]["partials"] for k in range(NCORES)]
    return _combine(partials)


# revision 24
# speedup vs baseline: 1.3337x; 1.0191x over previous
"""Trainium2 Bass kernel for nn_DentalAnatomyLoss (v2).

Computes, for segmentation [B=2, C=32, D=64, H=128, W=128] fp32:
  - crown/root ratio loss (per (b,c) sums over d<32 / d>=32)
  - 3D total-variation loss (mean |diff| along w, h, d)
  - returns stack([crown_root, smoothness, total_anatomy]) fp32 [3]

Pure data-parallel over the 64 (b,c) slices, 8 per NeuronCore. Each
core reduces its 32 MiB shard to a [128, ACC] fp32 partial tensor; the
host combines partials into the 3 scalars.

Layout: d-on-partitions, 2 slices per chunk pair (cp): partition
p = s*64 + d; free = (h, w) = 16384 bf16 (fp32->bf16 cast inside the
SWDGE DMA). sum|a-b| = 2*sum(max(a,b)) - sum(a) - sum(b); the signed
sums telescope to per-plane/edge sums.

Engine assignment (HW-measured rates; DVE tensor_tensor runs 2x_1p for
bf16 but TensorScalarPtrReduce/STT only 1x; per-DVE-op overhead
~0.8us, so ops are half-cp sized):
  VectorE: gy max via tensor_tensor(max) (2x, ~41us) + gx either as
    fused scalar_tensor_tensor max+accum (1x, no reduce stream; even
    cps) or tensor_tensor(max) into scratch (2x; odd cps) -> ~100us.
  TensorE: block-bidiag d-diff matmuls into PSUM + accumulating
    ones-row reduce matmuls of the gy (and odd-cp gx) scratches +
    [128,4] crown/root group-selector matmul -> ~95us.
  ScalarE: PSUM |dz| Abs+accum drains, tiny edge row/col sums, per-cp
    drains of the accumulating PSUM rows -> ~78us.
  DMA: ~109us/core measured floor (HBM ~358 GB/s/core) -> the target.

gy pairs of half 0 read w elements into half 1 (emitted after that
DMA), so no boundary ops and no gyb telescope terms.
"""

import os

import numpy as np

B, C, D, H, W = 2, 32, 64, 128, 128
NCORES = 8
JPC = (B * C) // NCORES  # slices per core
CROWN_ROOT_W = 2.0
SMOOTH_W = 1.5
EXPECTED_RATIO = 1.2

NQ = 4  # DMA quarters per chunk pair
GRP = (3, 3, 2)  # diff-PSUM group sizes (blocks) per quarter
GY_ROW = 32  # PSUM partition row of the gy-reduce accumulation
GX_ROW = 64  # PSUM partition row of the gx-reduce accumulation (odd cps)

_PROG_CACHE: dict = {}
last_exec_time_ns = None


def _layout(ncp):
    """acc column layout for the [128, acc_cols] fp32 partial tensor."""
    ndr = len(GRP) * NQ  # diff drains per cp
    col_mx = 0  # ncp*nq: per-plane sum(max w-pairs), per quarter
    col_r = col_mx + ncp * NQ  # ncp: per-plane sum(row0 + row_{h-1})
    col_c = col_r + ncp  # ncp: per-plane sum(col0 + col_{w-1})
    col_ps = col_c + ncp  # ncp: rows 0..3 = crown/root sums, row 32 = My
    col_dz = col_ps + ncp  # ncp*ndr: PSUM |dz| drains
    acc_cols = col_dz + ncp * ndr
    return ndr, col_mx, col_r, col_c, col_ps, col_dz, acc_cols


def _build_program(jpc=JPC, d=D, h=H, w=W, repeat=1, skip=()):
    """Build the (single) SPMD Bass program run identically on all cores.

    repeat>1 wraps the whole compute in a hardware For_i loop (identical
    result, used only for wall-clock timing of the kernel body).
    skip=(...) ablates op groups for engine-time attribution.
    """
    from contextlib import ExitStack

    import concourse.tile as tile
    from concourse import bacc, mybir

    f32 = mybir.dt.float32
    bf16 = mybir.dt.bfloat16
    AO = mybir.AluOpType
    AF = mybir.ActivationFunctionType

    ncp = jpc // 2
    P = 2 * d  # partitions per chunk pair
    fsz = h * w  # free size per partition (one (h,w) plane)
    qsz = fsz // NQ
    hq = h // NQ  # h-rows per quarter
    nblk = fsz // 512  # 512-col matmul blocks per cp
    bq = nblk // NQ  # blocks per quarter
    assert sum(GRP) == bq
    ndr, col_mx, col_r, col_c, col_ps, col_dz, acc_cols = _layout(ncp)

    nc = bacc.Bacc(
        "TRN2",
        target_bir_lowering=False,
        debug=False,
        enable_asserts=False,
        num_devices=NCORES,
    )
    seg = nc.dram_tensor("seg", [jpc, d, h, w], f32, kind="ExternalInput").ap()
    aux = nc.dram_tensor("aux", [P, P + 5], bf16, kind="ExternalInput").ap()
    out = nc.dram_tensor("partials", [P, acc_cols], f32, kind="ExternalOutput").ap()

    with tile.TileContext(nc) as tc, ExitStack() as ctx:
        singles = ctx.enter_context(tc.tile_pool(name="singles", bufs=1))
        xbp = ctx.enter_context(tc.tile_pool(name="xb", bufs=3))
        gyp = ctx.enter_context(tc.tile_pool(name="gy", bufs=2))
        gxp = ctx.enter_context(tc.tile_pool(name="gx", bufs=2))
        psp = ctx.enter_context(tc.tile_pool(name="ps", bufs=2, space="PSUM"))
        accp = ctx.enter_context(tc.tile_pool(name="accps", bufs=2, space="PSUM"))

        aux_sb = singles.tile([P, P + 5], bf16)
        nc.sync.dma_start(out=aux_sb, in_=aux)
        bd_ap = aux_sb[:, 0:P]  # block-bidiag d-diff lhsT
        xsel_ap = aux_sb[:, P : P + 4]  # crown/root group selector lhsT
        ones_ap = aux_sb[:, P + 4 : P + 5]  # gy-reduce lhsT

        acc = singles.tile([P, acc_cols], f32)
        nc.vector.memset(acc, 0.0)
        dump = singles.tile([P, (h // 2) * (w - 1)], bf16)  # VectorE out sink
        sdump = singles.tile([P, 512], bf16)  # ScalarE psum-drain out sink
        dummy = singles.tile([P, 1], bf16)  # ScalarE broadcast out sink

        def emit_gyred(c, half, gy_t, fd, acc_ps):
            # free-axis reduce of the gy max scratch: accumulate column
            # sums into PSUM row GY_ROW across all chunks of the cp.
            nchunks = (fd + 511) // 512
            for k in range(nchunks):
                c0 = k * 512
                csz = min(512, fd - c0)
                nc.tensor.matmul(
                    acc_ps[GY_ROW : GY_ROW + 1, 0:csz],
                    ones_ap,
                    gy_t[:, c0 : c0 + csz],
                    start=(half == 0 and k == 0),
                    stop=(half == 1 and k == nchunks - 1),
                    skip_group_check=True,
                )

        def cp_body(c):
            xb = xbp.tile([P, fsz], bf16)
            src = seg[2 * c : 2 * c + 2].rearrange("s d h w -> (s d) (h w)")
            xb3 = xb.rearrange("p (r c2) -> p r c2", c2=w)
            acc_ps = accp.tile([P, 512], f32)
            gy_tiles = []
            gx_tiles = []

            hsz = fsz // 2

            def emit_gy(half):
                # h-pairs for rows of this half-cp; half 0 reads w elements
                # into the next half (its DMA precedes this op).
                fd = hsz if half == 0 else hsz - w
                gy_t = gyp.tile([P, hsz], bf16)
                nc.vector.tensor_max(
                    gy_t[:, 0:fd],
                    xb[:, half * hsz : half * hsz + fd],
                    xb[:, half * hsz + w : half * hsz + w + fd],
                )
                gy_tiles.append((gy_t, fd))

            # gx route: fused STT on V everywhere. (Routing odd cps via
            # tensor_max + a TensorE ones-reduce chain was measured SLOWER
            # -- the extra T stream serializes against the diff matmuls.)
            gx_on_t = False

            def emit_gx(half):
                r0 = half * (h // 2)
                if not gx_on_t:
                    # fused max+accum over w-pairs (1x STT on V, no reduce)
                    colq = col_mx + NQ * c + half
                    nc.vector.scalar_tensor_tensor(
                        out=dump.rearrange("p (r c2) -> p r c2", c2=w - 1)[
                            :, 0 : h // 2, :
                        ],
                        in0=xb3[:, r0 : r0 + h // 2, 1:w],
                        scalar=0.0,
                        in1=xb3[:, r0 : r0 + h // 2, 0 : w - 1],
                        op0=AO.bypass,
                        op1=AO.max,
                        accum_out=acc[:, colq : colq + 1],
                    )
                else:
                    # 2x tensor_max into scratch; TensorE ones-matmul reduce
                    gx_t = gxp.tile([P, (h // 2) * (w - 1)], bf16)
                    nc.vector.tensor_max(
                        gx_t.rearrange("p (r c2) -> p r c2", c2=w - 1)[:, :, :],
                        xb3[:, r0 : r0 + h // 2, 0 : w - 1],
                        xb3[:, r0 : r0 + h // 2, 1:w],
                    )
                    gx_tiles.append((gx_t, (h // 2) * (w - 1)))

            def emit_gxred(half, gx_t, fd):
                nchunks = (fd + 511) // 512
                for k in range(nchunks):
                    c0 = k * 512
                    csz = min(512, fd - c0)
                    nc.tensor.matmul(
                        acc_ps[GX_ROW : GX_ROW + 1, 0:csz],
                        ones_ap,
                        gx_t[:, c0 : c0 + csz],
                        start=(half == 0 and k == 0),
                        stop=(half == 1 and k == nchunks - 1),
                        skip_group_check=True,
                    )

            for q in range(NQ):
                if "dma" not in skip:
                    nc.gpsimd.dma_start(
                        out=xb[:, q * qsz : (q + 1) * qsz],
                        in_=src[:, q * qsz : (q + 1) * qsz],
                    )
                elif q == 0:
                    # timing ablation: tiny write so the tile allocates
                    nc.gpsimd.dma_start(out=xb[:, 0:512], in_=src[:, 0:512])
                # --- VectorE: half-cp ops; gx fused STT, gy tt-max (2x) ---
                if "gx" not in skip and q in (1, NQ - 1):
                    emit_gx(0 if q == 1 else 1)
                if "gy" not in skip and q >= 2:
                    emit_gy(0 if q == 2 else 1)
                # --- TensorE: d-diff + group-sum matmuls; ScalarE drains ---
                for g, gsz in enumerate(GRP):
                    goff = q * bq + sum(GRP[:g])
                    if "gz" not in skip:
                        # constant-size PSUM tile (uniform pool slots); the
                        # last group only uses gsz of the GRP[0] blocks
                        ps = psp.tile([P, GRP[0], 512], f32)
                        for j in range(gsz):
                            blk = goff + j
                            nc.tensor.matmul(
                                ps[:, j, :],
                                bd_ap,
                                xb[:, blk * 512 : (blk + 1) * 512],
                                start=True,
                                stop=True,
                            )
                    if "xs" not in skip:
                        for j in range(gsz):
                            blk = goff + j
                            nc.tensor.matmul(
                                acc_ps[0:4, :],
                                xsel_ap,
                                xb[:, blk * 512 : (blk + 1) * 512],
                                start=(blk == 0),
                                stop=(blk == nblk - 1),
                                skip_group_check=True,
                            )
                    if "gz" not in skip and "drain" not in skip:
                        colx = col_dz + ndr * c + len(GRP) * q + g
                        nc.scalar.activation(
                            out=dummy.broadcast_to((P, gsz, 512)),
                            in_=ps[:, 0:gsz, :],
                            func=AF.Abs,
                            accum_out=acc[:, colx : colx + 1],
                        )
                # --- TensorE: scratch reduces (after the V max ops) ---
                if "gy" not in skip and "gyred" not in skip and q == NQ - 1:
                    emit_gyred(c, 0, *gy_tiles[0], acc_ps)
                    emit_gyred(c, 1, *gy_tiles[1], acc_ps)
                if "gx" not in skip and gx_on_t and q == NQ - 1:
                    emit_gxred(0, *gx_tiles[0])
                    emit_gxred(1, *gx_tiles[1])

            # --- ScalarE: edge sums + accumulating-PSUM drains ---
            if "edges" not in skip:
                rows = xb3[:, 0 : h : h - 1, :]
                nc.scalar.activation(
                    out=dummy.broadcast_to((P, 2, w)),
                    in_=rows,
                    func=AF.Copy,
                    accum_out=acc[:, col_r + c : col_r + c + 1],
                )
                colsv = xb.rearrange("p (r c2) -> p c2 r", c2=w)[:, 0 : w : w - 1, :]
                nc.scalar.activation(
                    out=dummy.broadcast_to((P, 2, h)),
                    in_=colsv,
                    func=AF.Copy,
                    accum_out=acc[:, col_c + c : col_c + c + 1],
                )
            if "xs" not in skip:
                nc.scalar.activation(
                    out=sdump[0:4, 0:512],
                    in_=acc_ps[0:4, :],
                    func=AF.Copy,
                    accum_out=acc[0:4, col_ps + c : col_ps + c + 1],
                )
            if "gy" not in skip and "gyred" not in skip:
                nc.scalar.activation(
                    out=sdump[GY_ROW : GY_ROW + 1, 0:512],
                    in_=acc_ps[GY_ROW : GY_ROW + 1, :],
                    func=AF.Copy,
                    accum_out=acc[GY_ROW : GY_ROW + 1, col_ps + c : col_ps + c + 1],
                )
            if "gx" not in skip and gx_on_t:
                nc.scalar.activation(
                    out=sdump[GX_ROW : GX_ROW + 1, 0:512],
                    in_=acc_ps[GX_ROW : GX_ROW + 1, :],
                    func=AF.Copy,
                    accum_out=acc[GX_ROW : GX_ROW + 1, col_ps + c : col_ps + c + 1],
                )

        def all_cps():
            for c in range(ncp):
                cp_body(c)

        if repeat == 1:
            all_cps()
        else:
            with tc.For_i(0, repeat, 1):
                all_cps()
        nc.sync.dma_start(out=out, in_=acc)

    nc.compile()
    return nc


def _get_program():
    key = "full"
    if key not in _PROG_CACHE:
        _PROG_CACHE[key] = _build_program()
    return _PROG_CACHE[key]


def _aux_np(d=D):
    """[2d, 2d+5] bf16 lhsT bundle: cols 0..2d-1 block-bidiag (out row m =
    x[m+1]-x[m] within each slice; cols d-1, 2d-1 zero), cols 2d..2d+3
    crown/root group selectors, col 2d+4 ones (gy reduce)."""
    import ml_dtypes

    P = 2 * d
    a = np.zeros((P, P + 5), dtype=np.float32)
    for col in range(P - 1):
        if col == d - 1:
            continue
        a[col, col] = -1.0
        a[col + 1, col] = 1.0
    hd = d // 2
    for j in range(4):
        a[j * hd : (j + 1) * hd, P + j] = 1.0
    a[:, P + 4] = 1.0
    return a.astype(ml_dtypes.bfloat16)


def _combine(partials, jpc=JPC, d=D, h=H, w=W):
    """Host-side finish: per-core [2d, acc_cols] fp32 partials -> [3]."""
    ncp = jpc // 2
    ndr, col_mx, col_r, col_c, col_ps, col_dz, acc_cols = _layout(ncp)

    nslice = jpc * len(partials)
    crown = np.zeros(nslice, dtype=np.float64)
    root = np.zeros(nslice, dtype=np.float64)
    gxy_sum = 0.0
    gz_sum = 0.0
    for k, p in enumerate(partials):
        p = p.astype(np.float64)
        for c in range(ncp):
            cr0, rt0, cr1, rt1 = p[0:4, col_ps + c]
            my = p[GY_ROW, col_ps + c]
            s_cp = cr0 + rt0 + cr1 + rt1
            r_cp = p[:, col_r + c].sum()
            c_cp = p[:, col_c + c].sum()
            # even cps: STT accum cols; odd cps: TensorE-reduce PSUM row
            mx = p[:, col_mx + NQ * c : col_mx + NQ * c + 2].sum()
            mx += p[GX_ROW, col_ps + c]
            # sum|a-b| = 2*sum(max) - sum(a) - sum(b); signed sums telescope
            gxy_sum += 2.0 * my - 2.0 * s_cp + r_cp
            gxy_sum += 2.0 * mx - 2.0 * s_cp + c_cp
            sl = k * jpc + 2 * c
            crown[sl], root[sl] = cr0, rt0
            crown[sl + 1], root[sl + 1] = cr1, rt1
        # diff rows d-1 and 2d-1 are |0| = 0 (zeroed bidiag columns)
        gz_sum += p[:, col_dz : col_dz + ncp * ndr].sum()

    total = crown + root
    valid = (total > 0) & (root > 0)
    safe_root = np.where(root > 0, root, 1.0)
    ratio_loss = np.where(valid, (crown / safe_root - EXPECTED_RATIO) ** 2, 0.0)
    cr_loss = ratio_loss.sum() / nslice

    nxy = nslice * d * h * (w - 1)  # == nslice * d * (h-1) * w
    nz = nslice * (d - 1) * h * w
    tv = gxy_sum / nxy + gz_sum / nz

    crown_root = cr_loss * CROWN_ROOT_W
    smoothness = tv * SMOOTH_W
    return np.array(
        [crown_root, smoothness, crown_root + smoothness], dtype=np.float32
    )


def kernel(segmentation: np.ndarray) -> np.ndarray:
    global last_exec_time_ns
    from concourse.bass_utils import run_bass_kernel_spmd

    seg = np.ascontiguousarray(np.asarray(segmentation), dtype=np.float32)
    assert seg.shape == (B, C, D, H, W)
    nc = _get_program()

    aux = _aux_np()
    shards = seg.reshape(B * C, D, H, W)
    in_maps = [
        {"seg": np.ascontiguousarray(shards[k * JPC : (k + 1) * JPC]), "aux": aux}
        for k in range(NCORES)
    ]
    trace = bool(os.environ.get("BASS_TRACE"))
    res = run_bass_kernel_spmd(nc, in_maps, list(range(NCORES)), trace=trace)
    last_exec_time_ns = res.exec_time_ns
    partials = [res.results[k]["partials"] for k in range(NCORES)]
    return _combine(partials)


# revision 32
# speedup vs baseline: 1.3640x; 1.0227x over previous
"""Trainium2 Bass kernel for nn_DentalAnatomyLoss (v2).

Computes, for segmentation [B=2, C=32, D=64, H=128, W=128] fp32:
  - crown/root ratio loss (per (b,c) sums over d<32 / d>=32)
  - 3D total-variation loss (mean |diff| along w, h, d)
  - returns stack([crown_root, smoothness, total_anatomy]) fp32 [3]

Pure data-parallel over the 64 (b,c) slices, 8 per NeuronCore. Each
core reduces its 32 MiB shard to a [128, ACC] fp32 partial tensor; the
host combines partials into the 3 scalars.

Layout: d-on-partitions, 2 slices per chunk pair (cp): partition
p = s*64 + d; free = (h, w) = 16384 bf16 (fp32->bf16 cast inside the
SWDGE DMA). sum|a-b| = 2*sum(max(a,b)) - sum(a) - sum(b); the signed
sums telescope to per-plane/edge sums.

Engine assignment (HW-measured rates; DVE tensor_tensor runs 2x_1p for
bf16 but TensorScalarPtrReduce/STT only 1x; per-DVE-op overhead
~0.8us, so ops are half-cp sized):
  VectorE: gy max via tensor_tensor(max) (2x, ~41us) + gx either as
    fused scalar_tensor_tensor max+accum (1x, no reduce stream; even
    cps) or tensor_tensor(max) into scratch (2x; odd cps) -> ~100us.
  TensorE: block-bidiag d-diff matmuls into PSUM + accumulating
    ones-row reduce matmuls of the gy (and odd-cp gx) scratches +
    [128,4] crown/root group-selector matmul -> ~95us.
  ScalarE: PSUM |dz| Abs+accum drains, tiny edge row/col sums, per-cp
    drains of the accumulating PSUM rows -> ~78us.
  DMA: ~109us/core measured floor (HBM ~358 GB/s/core) -> the target.

gy pairs of half 0 read w elements into half 1 (emitted after that
DMA), so no boundary ops and no gyb telescope terms.
"""

import os

import numpy as np

B, C, D, H, W = 2, 32, 64, 128, 128
NCORES = 8
JPC = (B * C) // NCORES  # slices per core
CROWN_ROOT_W = 2.0
SMOOTH_W = 1.5
EXPECTED_RATIO = 1.2

NQ = 4  # DMA quarters per chunk pair
GRP = (3, 3, 2)  # diff-PSUM group sizes (blocks) per quarter
GY_ROW = 32  # PSUM partition row of the gy-reduce accumulation
GX_ROW = 64  # PSUM partition row of the gx-reduce accumulation (odd cps)

_PROG_CACHE: dict = {}
last_exec_time_ns = None


def _layout(ncp):
    """acc column layout for the [128, acc_cols] fp32 partial tensor."""
    ndr = len(GRP) * NQ  # diff drains per cp
    col_mx = 0  # ncp*nq: per-plane sum(max w-pairs), per quarter
    col_r = col_mx + ncp * NQ  # ncp: per-plane sum(row0 + row_{h-1})
    col_c = col_r + ncp  # ncp: per-plane sum(col0 + col_{w-1})
    col_ps = col_c + ncp  # ncp: rows 0..3 = crown/root sums, row 32 = My
    col_dz = col_ps + ncp  # ncp*ndr: PSUM |dz| drains
    acc_cols = col_dz + ncp * ndr
    return ndr, col_mx, col_r, col_c, col_ps, col_dz, acc_cols


def _build_program(jpc=JPC, d=D, h=H, w=W, repeat=1, skip=()):
    """Build the (single) SPMD Bass program run identically on all cores.

    repeat>1 wraps the whole compute in a hardware For_i loop (identical
    result, used only for wall-clock timing of the kernel body).
    skip=(...) ablates op groups for engine-time attribution.
    """
    from contextlib import ExitStack

    import concourse.tile as tile
    from concourse import bacc, mybir

    f32 = mybir.dt.float32
    bf16 = mybir.dt.bfloat16
    AO = mybir.AluOpType
    AF = mybir.ActivationFunctionType

    ncp = jpc // 2
    P = 2 * d  # partitions per chunk pair
    fsz = h * w  # free size per partition (one (h,w) plane)
    qsz = fsz // NQ
    hq = h // NQ  # h-rows per quarter
    nblk = fsz // 512  # 512-col matmul blocks per cp
    bq = nblk // NQ  # blocks per quarter
    assert sum(GRP) == bq
    ndr, col_mx, col_r, col_c, col_ps, col_dz, acc_cols = _layout(ncp)

    nc = bacc.Bacc(
        "TRN2",
        target_bir_lowering=False,
        debug=False,
        enable_asserts=False,
        num_devices=NCORES,
    )
    seg = nc.dram_tensor("seg", [jpc, d, h, w], f32, kind="ExternalInput").ap()
    aux = nc.dram_tensor("aux", [P, P + 5], bf16, kind="ExternalInput").ap()
    out = nc.dram_tensor("partials", [P, acc_cols], f32, kind="ExternalOutput").ap()

    with tile.TileContext(nc) as tc, ExitStack() as ctx:
        singles = ctx.enter_context(tc.tile_pool(name="singles", bufs=1))
        xbp = ctx.enter_context(tc.tile_pool(name="xb", bufs=3))
        gyp = ctx.enter_context(tc.tile_pool(name="gy", bufs=2))
        psp = ctx.enter_context(tc.tile_pool(name="ps", bufs=2, space="PSUM"))
        accp = ctx.enter_context(tc.tile_pool(name="accps", bufs=2, space="PSUM"))

        aux_sb = singles.tile([P, P + 5], bf16)
        nc.sync.dma_start(out=aux_sb, in_=aux)
        bd_ap = aux_sb[:, 0:P]  # block-bidiag d-diff lhsT
        xsel_ap = aux_sb[:, P : P + 4]  # crown/root group selector lhsT
        ones_ap = aux_sb[:, P + 4 : P + 5]  # gy-reduce lhsT

        acc = singles.tile([P, acc_cols], f32)
        nc.vector.memset(acc, 0.0)
        sdump = singles.tile([P, 512], bf16)  # ScalarE psum-drain out sink
        dummy = singles.tile([P, 1], bf16)  # ScalarE broadcast out sink
        vdummy = singles.tile([P, 1], bf16)  # VectorE broadcast out sink

        def emit_gyred(c, gy_t, fd, acc_ps):
            # free-axis reduce of the gy max scratch: accumulate column
            # sums into PSUM row GY_ROW across all chunks of the cp.
            nchunks = (fd + 511) // 512
            for k in range(nchunks):
                c0 = k * 512
                csz = min(512, fd - c0)
                nc.tensor.matmul(
                    acc_ps[GY_ROW : GY_ROW + 1, 0:csz],
                    ones_ap,
                    gy_t[:, c0 : c0 + csz],
                    start=(k == 0),
                    stop=(k == nchunks - 1),
                    skip_group_check=True,
                )

        def cp_body(c):
            xb = xbp.tile([P, fsz], bf16)
            src = seg[2 * c : 2 * c + 2].rearrange("s d h w -> (s d) (h w)")
            xb3 = xb.rearrange("p (r c2) -> p r c2", c2=w)
            acc_ps = accp.tile([P, 512], f32)
            gy_tiles = []

            def emit_gy():
                # one full-cp op: h-pairs rows 0..h-2 (amortizes the
                # ~0.8us/op DVE overhead over the largest possible FD)
                fd = fsz - w
                gy_t = gyp.tile([P, fsz - w], bf16)
                nc.vector.tensor_max(
                    gy_t[:, 0:fd], xb[:, 0:fd], xb[:, w : w + fd]
                )
                gy_tiles.append((gy_t, fd))

            def emit_gx():
                # one full-cp fused max+accum over w-pairs (1x STT on V, no
                # reduce stream; broadcast out avoids a 4MB scratch).
                # (Routing gx via tensor_max + a TensorE ones-reduce chain
                # was measured SLOWER -- the extra T stream serializes
                # against the diff matmuls.)
                colq = col_mx + NQ * c
                nc.vector.scalar_tensor_tensor(
                    out=vdummy.broadcast_to((P, h, w - 1)),
                    in0=xb3[:, :, 1:w],
                    scalar=0.0,
                    in1=xb3[:, :, 0 : w - 1],
                    op0=AO.bypass,
                    op1=AO.max,
                    accum_out=acc[:, colq : colq + 1],
                )

            for q in range(NQ):
                if "dma" not in skip:
                    nc.gpsimd.dma_start(
                        out=xb[:, q * qsz : (q + 1) * qsz],
                        in_=src[:, q * qsz : (q + 1) * qsz],
                    )
                elif q == 0:
                    # timing ablation: tiny write so the tile allocates
                    nc.gpsimd.dma_start(out=xb[:, 0:512], in_=src[:, 0:512])
                # --- VectorE: full-cp ops after the last DMA; gy (2x
                # tensor_max) first so TensorE's gyred overlaps the gx STT
                if q == NQ - 1:
                    if "gy" not in skip:
                        emit_gy()
                    if "gx" not in skip:
                        emit_gx()
                # --- TensorE: d-diff + group-sum matmuls; ScalarE drains ---
                for g, gsz in enumerate(GRP):
                    goff = q * bq + sum(GRP[:g])
                    if "gz" not in skip:
                        # constant-size PSUM tile (uniform pool slots); the
                        # last group only uses gsz of the GRP[0] blocks
                        ps = psp.tile([P, GRP[0], 512], f32)
                        for j in range(gsz):
                            blk = goff + j
                            nc.tensor.matmul(
                                ps[:, j, :],
                                bd_ap,
                                xb[:, blk * 512 : (blk + 1) * 512],
                                start=True,
                                stop=True,
                            )
                    if "xs" not in skip:
                        for j in range(gsz):
                            blk = goff + j
                            nc.tensor.matmul(
                                acc_ps[0:4, :],
                                xsel_ap,
                                xb[:, blk * 512 : (blk + 1) * 512],
                                start=(blk == 0),
                                stop=(blk == nblk - 1),
                                skip_group_check=True,
                            )
                    if "gz" not in skip and "drain" not in skip:
                        colx = col_dz + ndr * c + len(GRP) * q + g
                        nc.scalar.activation(
                            out=dummy.broadcast_to((P, gsz, 512)),
                            in_=ps[:, 0:gsz, :],
                            func=AF.Abs,
                            accum_out=acc[:, colx : colx + 1],
                        )
                # --- TensorE: gy scratch reduce (after the V max op) ---
                if "gy" not in skip and "gyred" not in skip and q == NQ - 1:
                    emit_gyred(c, *gy_tiles[0], acc_ps)

            # --- ScalarE: edge sums + accumulating-PSUM drains ---
            if "edges" not in skip:
                rows = xb3[:, 0 : h : h - 1, :]
                nc.scalar.activation(
                    out=dummy.broadcast_to((P, 2, w)),
                    in_=rows,
                    func=AF.Copy,
                    accum_out=acc[:, col_r + c : col_r + c + 1],
                )
                colsv = xb.rearrange("p (r c2) -> p c2 r", c2=w)[:, 0 : w : w - 1, :]
                nc.scalar.activation(
                    out=dummy.broadcast_to((P, 2, h)),
                    in_=colsv,
                    func=AF.Copy,
                    accum_out=acc[:, col_c + c : col_c + c + 1],
                )
            if "xs" not in skip:
                nc.scalar.activation(
                    out=sdump[0:4, 0:512],
                    in_=acc_ps[0:4, :],
                    func=AF.Copy,
                    accum_out=acc[0:4, col_ps + c : col_ps + c + 1],
                )
            if "gy" not in skip and "gyred" not in skip:
                nc.scalar.activation(
                    out=sdump[GY_ROW : GY_ROW + 1, 0:512],
                    in_=acc_ps[GY_ROW : GY_ROW + 1, :],
                    func=AF.Copy,
                    accum_out=acc[GY_ROW : GY_ROW + 1, col_ps + c : col_ps + c + 1],
                )


        def all_cps():
            for c in range(ncp):
                cp_body(c)

        if repeat == 1:
            all_cps()
        else:
            with tc.For_i(0, repeat, 1):
                all_cps()
        nc.sync.dma_start(out=out, in_=acc)

    nc.compile()
    return nc


def _get_program():
    key = "full"
    if key not in _PROG_CACHE:
        _PROG_CACHE[key] = _build_program()
    return _PROG_CACHE[key]


def _aux_np(d=D):
    """[2d, 2d+5] bf16 lhsT bundle: cols 0..2d-1 block-bidiag (out row m =
    x[m+1]-x[m] within each slice; cols d-1, 2d-1 zero), cols 2d..2d+3
    crown/root group selectors, col 2d+4 ones (gy reduce)."""
    import ml_dtypes

    P = 2 * d
    a = np.zeros((P, P + 5), dtype=np.float32)
    for col in range(P - 1):
        if col == d - 1:
            continue
        a[col, col] = -1.0
        a[col + 1, col] = 1.0
    hd = d // 2
    for j in range(4):
        a[j * hd : (j + 1) * hd, P + j] = 1.0
    a[:, P + 4] = 1.0
    return a.astype(ml_dtypes.bfloat16)


def _combine(partials, jpc=JPC, d=D, h=H, w=W):
    """Host-side finish: per-core [2d, acc_cols] fp32 partials -> [3]."""
    ncp = jpc // 2
    ndr, col_mx, col_r, col_c, col_ps, col_dz, acc_cols = _layout(ncp)

    nslice = jpc * len(partials)
    crown = np.zeros(nslice, dtype=np.float64)
    root = np.zeros(nslice, dtype=np.float64)
    gxy_sum = 0.0
    gz_sum = 0.0
    for k, p in enumerate(partials):
        p = p.astype(np.float64)
        for c in range(ncp):
            cr0, rt0, cr1, rt1 = p[0:4, col_ps + c]
            my = p[GY_ROW, col_ps + c]
            s_cp = cr0 + rt0 + cr1 + rt1
            r_cp = p[:, col_r + c].sum()
            c_cp = p[:, col_c + c].sum()
            mx = p[:, col_mx + NQ * c].sum()
            # sum|a-b| = 2*sum(max) - sum(a) - sum(b); signed sums telescope
            gxy_sum += 2.0 * my - 2.0 * s_cp + r_cp
            gxy_sum += 2.0 * mx - 2.0 * s_cp + c_cp
            sl = k * jpc + 2 * c
            crown[sl], root[sl] = cr0, rt0
            crown[sl + 1], root[sl + 1] = cr1, rt1
        # diff rows d-1 and 2d-1 are |0| = 0 (zeroed bidiag columns)
        gz_sum += p[:, col_dz : col_dz + ncp * ndr].sum()

    total = crown + root
    valid = (total > 0) & (root > 0)
    safe_root = np.where(root > 0, root, 1.0)
    ratio_loss = np.where(valid, (crown / safe_root - EXPECTED_RATIO) ** 2, 0.0)
    cr_loss = ratio_loss.sum() / nslice

    nxy = nslice * d * h * (w - 1)  # == nslice * d * (h-1) * w
    nz = nslice * (d - 1) * h * w
    tv = gxy_sum / nxy + gz_sum / nz

    crown_root = cr_loss * CROWN_ROOT_W
    smoothness = tv * SMOOTH_W
    return np.array(
        [crown_root, smoothness, crown_root + smoothness], dtype=np.float32
    )


def kernel(segmentation: np.ndarray) -> np.ndarray:
    global last_exec_time_ns
    from concourse.bass_utils import run_bass_kernel_spmd

    seg = np.ascontiguousarray(np.asarray(segmentation), dtype=np.float32)
    assert seg.shape == (B, C, D, H, W)
    nc = _get_program()

    aux = _aux_np()
    shards = seg.reshape(B * C, D, H, W)
    in_maps = [
        {"seg": np.ascontiguousarray(shards[k * JPC : (k + 1) * JPC]), "aux": aux}
        for k in range(NCORES)
    ]
    trace = bool(os.environ.get("BASS_TRACE"))
    res = run_bass_kernel_spmd(nc, in_maps, list(range(NCORES)), trace=trace)
    last_exec_time_ns = res.exec_time_ns
    partials = [res.results[k]["partials"] for k in range(NCORES)]
    return _combine(partials)
